# revision 8
# baseline (speedup 1.0000x reference)
"""Causal attention head (B=16, S=2048, d=64) on 8 TRN2 NeuronCores.

Data parallel over batch: each core gets 2 batches and computes its full
S x S causal attention.

Per-core algorithm (transposed-scores layout):
  scores_T[j, i] = sum_d k[j,d] q[i,d] / 64      (j on PSUM partitions)
  attn_T = exp(scores_T)  (scores are tiny: |s|<1, so no max-subtraction)
  out[i, 0:64] | l[i] = sum_j attn_T[j, i] * [v[j, :] | 1]
    computed per (j-chunk, i-chunk) 128x128 block with attn_T as the
    STATIONARY operand and [v|1] (65 cols) as the moving operand -> out in
    natural [i, d] layout, 65 moving cycles per block.
  out[i] /= l[i]
Causality: blocks with jc > ic are never computed (exp windows trimmed);
the 4 diagonal 128x128 blocks per i-tile are masked via affine_select.
"""

import numpy as np

import concourse.bacc as bacc
import concourse.bass as bass
import concourse.mybir as mybir
import concourse.tile as tile
from concourse.bass_utils import run_bass_kernel_spmd
from concourse.masks import make_identity

F32 = mybir.dt.float32
BF16 = mybir.dt.bfloat16

B, S, D = 16, 2048, 64
N_CORES = 8
BPC = B // N_CORES  # batches per core
P = 128
ITILE = 512               # i-tile width (free dim of scores_T)
N_IT = S // ITILE         # 4 i-tiles
N_JC = S // P             # 16 j-chunks
SCALE = 1.0 / D


import os as _os

QUAD_BUFS = int(_os.environ.get("K_QUAD_BUFS", "2"))
ACC_BUFS = int(_os.environ.get("K_ACC_BUFS", "2"))
TRP_BUFS = int(_os.environ.get("K_TRP_BUFS", "2"))
LAG_N = int(_os.environ.get("K_LAG", "4"))
INJ_N = int(_os.environ.get("K_INJ", "10"))
ATTN_BUFS = int(_os.environ.get("K_ATTN_BUFS", "4"))
NOPACK = int(_os.environ.get("K_NOPACK", "0"))  # timing-only A/B probe
MERGE = int(_os.environ.get("K_MERGE", "1"))  # one pipeline across batches


def build_kernel(loop: int = 0, level: int = 4):
    # level: probe ladder for benchmarking — 1: input DMA only, 2: + stage A,
    # 3: + mm1/exp/mask, 4: full kernel (default; the only correct one),
    # 5: stage A + all matmuls, no ACT/mask, 6: stage A + exp stream only
    nc = bacc.Bacc("TRN2", target_bir_lowering=False, debug=False)
    q_h = nc.dram_tensor("q", [BPC, S, D], F32, kind="ExternalInput").ap()
    k_h = nc.dram_tensor("k", [BPC, S, D], F32, kind="ExternalInput").ap()
    v_h = nc.dram_tensor("v", [BPC, S, D], F32, kind="ExternalInput").ap()
    o_h = nc.dram_tensor("o", [BPC, S, D], F32, kind="ExternalOutput").ap()

    with tile.TileContext(nc) as tc:
        with (
            tc.tile_pool(name="const", bufs=1) as const,
            tc.tile_pool(name="stage", bufs=2) as stage,
            tc.tile_pool(name="qkt", bufs=2) as qkt,
            tc.tile_pool(name="attn", bufs=ATTN_BUFS) as attnp,
            tc.tile_pool(name="outs", bufs=2) as outs,
            tc.tile_pool(name="quad", bufs=QUAD_BUFS, space="PSUM") as quadp,
            tc.tile_pool(name="acc", bufs=ACC_BUFS, space="PSUM") as accp,
            tc.tile_pool(name="trp", bufs=TRP_BUFS, space="PSUM") as trp,
        ):
            ident_f = const.tile([P, P], F32)
            make_identity(nc, ident_f)
            ident_b = const.tile([P, P], BF16)
            nc.vector.tensor_copy(ident_b, ident_f)
            # warm the ACT exp table while the input DMAs run
            warm = const.tile([P, 1], F32)
            nc.scalar.activation(
                warm, ident_f[:, 0:1], mybir.ActivationFunctionType.Exp
            )

            def stage_a_loads(b):
                # ---- stage inputs, natural layout [128, 16, 64]
                # halves so DMA/transpose pipeline at half granularity
                H = N_JC // 2
                qn = stage.tile([P, N_JC, D], F32, tag="qn", name=f"qn{b}")
                kn = stage.tile([P, N_JC, D], F32, tag="kn", name=f"kn{b}")
                vn = stage.tile([P, N_JC, D], F32, tag="vn", name=f"vn{b}")
                vp = stage.tile([P, N_JC, D + 1], BF16, tag="vp", name=f"vp{b}")
                kr = k_h[b].rearrange("(n p) d -> p n d", p=P)
                qr = q_h[b].rearrange("(n p) d -> p n d", p=P)
                vr = v_h[b].rearrange("(n p) d -> p n d", p=P)
                Q = N_JC // 4
                # quarters of k/q interleaved (transposes consume in this
                # order), v halves placed just before their first use
                sched = [
                    (kr, kn, 0), (qr, qn, 0), (kr, kn, 1), (qr, qn, 1),
                    (vr, vn, None), (kr, kn, 2), (qr, qn, 2),
                    (kr, kn, 3), (qr, qn, 3), (vr, vn, None),
                ]
                vh = 0
                for src, dst, qi in sched:
                    if qi is None:
                        sl = slice(H * vh, H * (vh + 1))
                        nc.sync.dma_start(dst[:, sl, :], src[:, sl, :])
                        if level >= 2:
                            nc.gpsimd.tensor_copy(vp[:, sl, 0:D], vn[:, sl, :])
                        vh += 1
                    else:
                        sl = slice(Q * qi, Q * (qi + 1))
                        nc.sync.dma_start(dst[:, sl, :], src[:, sl, :])
                if level >= 2:
                    nc.gpsimd.memset(vp[:, :, D : D + 1], 1.0)
                return qn, kn, vn, vp

            def stage_a_pe(b, qn, kn):
                """Return (qt, kt2, thunks): each thunk emits one PE
                transpose group; caller decides where to interleave them."""
                # K^T interleaved-pairs layout [128, S/2] bf16:
                #  kt2[0:64,  128e+s] = K^T of chunk 2e
                #  kt2[64:128,128e+s] = K^T of chunk 2e+1
                # (transposing a [128, 128] block of TWO adjacent chunks puts
                # chunk 2e on partitions 0:64 and chunk 2e+1 on 64:128).
                # Transposes read the fp32 tiles; bf16 cast is folded into
                # the PSUM->SBUF copy.
                kt2 = qkt.tile([P, S // 2], BF16, tag="kt", name=f"kt{b}")
                qt = qkt.tile([P, S], BF16, tag="qt", name=f"qt{b}")
                thunks = []

                def k_group(g):
                    tr = trp.tile([P, 2 * P], F32, tag="trp", name=f"trk{b}_{g}")
                    for u in range(2):
                        e = 2 * g + u
                        nc.tensor.transpose(
                            tr[:, P * u : P * (u + 1)],
                            kn[:, 2 * e : 2 * e + 2, :],
                            ident_f,
                        )
                    nc.vector.tensor_copy(
                        kt2[:, 2 * P * g : 2 * P * (g + 1)], tr
                    )

                def q_group(g):
                    # Q^T duplicated into both partition halves (two copies
                    # from the same PSUM tile; partition-shifted second copy)
                    tr = trp.tile([P, 4 * P], F32, tag="trp", name=f"trq{b}_{g}")
                    for u in range(4):
                        nc.tensor.transpose(
                            tr[0:D, P * u : P * (u + 1)],
                            qn[:, 4 * g + u, :],
                            ident_f,
                        )
                    sl = slice(4 * P * g, 4 * P * (g + 1))
                    nc.vector.tensor_copy(qt[0:D, sl], tr[0:D])
                    nc.vector.tensor_copy(qt[D : 2 * D, sl], tr[0:D])

                # interleave k/q groups so the data the first matmuls need
                # (low j-chunks, low i-columns) is ready earliest
                for g in range(4):
                    thunks.append(lambda g=g: k_group(g))
                    thunks.append(lambda g=g: q_group(g))
                return qt, kt2, thunks

            def one_pass():
                loaded = [stage_a_loads(b) for b in range(BPC)]
                if level < 2:
                    nc.sync.dma_start(o_h[0, 0:P, :], ident_f[:, 0:D])
                    return
                pe_stage = [
                    stage_a_pe(b, loaded[b][0], loaded[b][1])
                    for b in range(BPC)
                ]
                staged = []
                for b in range(BPC):
                    qt, kt2, thunks = pe_stage[b]
                    staged.append((qt, kt2, loaded[b][3], thunks))
                if level < 3:
                    for _, _, _, thunks in staged:
                        for t in thunks:
                            t()
                    nc.sync.dma_start(o_h[0, 0:P, :], ident_f[:, 0:D])
                    return
                LAG = LAG_N  # mm2 trails mm1/exp by LAG pairs: PE (strict FIFO)
                #          must never queue an mm2 whose exp isn't done yet
                pairs = [
                    (it, pr) for it in range(N_IT) for pr in range(2 * (it + 1))
                ]
                INJ_START = INJ_N  # during batch b's main loop, emit batch b+1's
                #                 PE transpose groups starting at this pair

                def stage_norm(b, it, out_ps):
                    # out_ps [128, 4, 65]: i = 128*k + p; normalize and store
                    rec = outs.tile([P, N_IT], F32, tag="rec")
                    nc.vector.reciprocal(rec, out_ps[:, :, D])
                    fin = outs.tile([P, N_IT, D], F32, tag="fin")
                    nc.vector.tensor_tensor(
                        fin,
                        out_ps[:, :, 0:D],
                        rec[:, :, None].to_broadcast((P, N_IT, D)),
                        mybir.AluOpType.mult,
                    )
                    r0 = ITILE * it
                    nc.sync.dma_start(
                        o_h[b, r0 : r0 + ITILE, :].rearrange(
                            "(s p) d -> p s d", p=P
                        ),
                        fin,
                    )

                # one software pipeline across BOTH batches: no drain/refill
                # at the batch boundary. Batch 1 runs its i-tiles in
                # DESCENDING order so the pipeline drains on the smallest
                # tile (it=0: shortest mm2+norm tail after the last exp).
                tile_order = {0: list(range(N_IT)), 1: list(range(N_IT))[::-1]}
                gpairs = [
                    (b, it, pr)
                    for b in range(BPC)
                    for it in tile_order[b]
                    for pr in range(2 * (it + 1))
                ]
                npb = len(pairs)
                atc = None
                if level == 5:
                    atc = attnp.tile([P, 2, ITILE], BF16, tag="attn")
                    nc.gpsimd.memset(atc, 0.25)
                out_ps_by_key = {}
                at_by_idx = {}

                def do_mm2(idx):
                    # the acc bank holds all 4 i-chunk accumulators as ONE
                    # psum accumulation group: start only on the very first
                    # matmul of the i-tile (zeroes the whole 2KB bank), stop
                    # only on the very last.
                    b, it, pr = gpairs[idx]
                    vp = staged[b][2]
                    at = at_by_idx.pop(idx)
                    out_ps = out_ps_by_key[(b, it)]
                    for c in range(2):
                        jc = 2 * pr + c
                        kj = jc - 4 * it  # relative j-chunk in this i-tile
                        for kc in range(max(kj, 0), N_IT):
                            nc.tensor.matmul(
                                out_ps[:, kc, 0 : D + 1],
                                lhsT=at[:, c, P * kc : P * (kc + 1)],
                                rhs=vp[:, jc, :],
                                start=(pr == 0 and c == 0 and kc == 0),
                                stop=(c == 1 and kc == N_IT - 1
                                      and pr == 2 * it + 1),
                            )
                    if pr == 2 * it + 1:
                        stage_norm(b, it, out_ps_by_key.pop((b, it)))

                # prefetch batch 0's first transpose groups before the loop
                for _i in range(2):
                    staged[0][3].pop(0)()
                for idx, (b, it, pr) in enumerate(gpairs):
                    qt, kt2, vp, _ = staged[b]
                    if pr == 0:
                        # padded to a full 2KB psum bank so pool slots
                        # never share a zero region
                        out_ps_by_key[(b, it)] = accp.tile(
                            [P, N_IT, P], F32, tag="acc",
                            name=f"acc{b}_{it}",
                        )
                    st = quadp.tile([P, 2, ITILE], F32, tag="quad")
                    isl = slice(ITILE * it, ITILE * (it + 1))
                    diag = pr >= 2 * it  # diagonal pair: windows trimmed
                    at = None
                    if level != 5:
                        at = attnp.tile([P, 2, ITILE], BF16, tag="attn")
                    for c in range(2):
                        kj = 2 * pr + c - 4 * it
                        i0 = P * kj if diag else 0
                        # c=0 in PE row group 0, c=1 in row group 64:
                        # the two matmuls stream concurrently
                        h = slice(0, D) if NOPACK else slice(D * c, D * (c + 1))
                        nc.tensor.matmul(
                            st[:, c, i0:ITILE],
                            lhsT=kt2[h, P * pr : P * (pr + 1)],
                            rhs=qt[h, ITILE * it + i0 : ITILE * (it + 1)],
                            start=True,
                            stop=True,
                        )
                    if level != 5:
                        # one exp per pair; diagonal pairs use a shared
                        # rectangular window [e0:512] for both c (cols below
                        # each chunk's own window hold pending-zero psum ->
                        # exp gives 1.0 there; mm2 never reads those blocks)
                        e0 = 2 * P * (pr - 2 * it) if diag else 0
                        nc.scalar.activation(
                            at[:, :, e0:ITILE], st[:, :, e0:ITILE],
                            mybir.ActivationFunctionType.Exp,
                            scale=SCALE,
                        )
                        if diag and level != 7:
                            for c in range(2):
                                # mask the diagonal 128x128 block: keep i>=j
                                i0 = P * (2 * pr + c - 4 * it)
                                nc.gpsimd.affine_select(
                                    out=at[:, c, i0 : i0 + P],
                                    in_=at[:, c, i0 : i0 + P],
                                    compare_op=mybir.AluOpType.is_ge,
                                    fill=0.0,
                                    base=0,
                                    pattern=[[1, P]],
                                    channel_multiplier=-1,
                                )
                    if b == 0 and pr == 2 * it + 1:
                        # prefetch the NEXT i-tile's k_group/q_group right
                        # after this tile's last mm1/exp is emitted
                        th = staged[0][3]
                        for _i in range(2):
                            if th:
                                th.pop(0)()
                    if level == 5:
                        at_by_idx[idx] = atc
                    else:
                        at_by_idx[idx] = at
                    if level == 3:
                        at_by_idx.pop(idx)
                        continue
                    if idx >= LAG and (MERGE or idx % npb >= LAG):
                        do_mm2(idx - LAG)
                    if not MERGE and idx % npb == npb - 1:
                        # flush this batch's lagged mm2s before the next batch
                        for j in range(idx - LAG + 1, idx + 1):
                            do_mm2(j)
                    # inject next batch's transpose groups into this batch's
                    # main loop (PE FIFO: their input DMAs are long done)
                    bi = idx // npb
                    if idx % npb >= INJ_START and bi + 1 < BPC:
                        th = staged[bi + 1][3]
                        if th:
                            th.pop(0)()
                if level != 3 and MERGE:
                    for idx in range(len(gpairs) - LAG, len(gpairs)):
                        do_mm2(idx)
                for b in range(BPC):
                    for t in staged[b][3]:
                        t()
                if level < 4:
                    nc.sync.dma_start(o_h[0, 0:P, :], ident_f[:, 0:D])

            if loop:
                hints = ()
                if _os.environ.get("K_LOOP_HINTS"):
                    hints = (
                        mybir.EngineType.PE,
                        mybir.EngineType.Activation,
                        mybir.EngineType.DVE,
                        mybir.EngineType.Pool,
                        mybir.EngineType.SP,
                    )
                with tc.For_i(0, loop, 1, hint_engines=hints):
                    one_pass()
            else:
                one_pass()

    nc.compile()
    return nc


_CACHE: dict = {}


def _get_nc(loop: int = 0):
    if loop not in _CACHE:
        _CACHE[loop] = build_kernel(loop)
    return _CACHE[loop]


def kernel(q: np.ndarray, k: np.ndarray, v: np.ndarray) -> np.ndarray:
    q = np.ascontiguousarray(q, dtype=np.float32)
    k = np.ascontiguousarray(k, dtype=np.float32)
    v = np.ascontiguousarray(v, dtype=np.float32)
    nc = _get_nc(0)
    in_maps = [
        {
            "q": q[BPC * i : BPC * (i + 1)],
            "k": k[BPC * i : BPC * (i + 1)],
            "v": v[BPC * i : BPC * (i + 1)],
        }
        for i in range(N_CORES)
    ]
    res = run_bass_kernel_spmd(nc, in_maps, list(range(N_CORES)))
    return np.concatenate([res.results[i]["o"] for i in range(N_CORES)], axis=0)


# revision 14
# speedup vs baseline: 1.1043x; 1.1043x over previous
"""Causal attention head (B=16, S=2048, d=64) on 8 TRN2 NeuronCores.

Data parallel over batch: each core gets 2 batches and computes its full
S x S causal attention.

Per-core algorithm (transposed-scores layout):
  scores_T[j, i] = sum_d k[j,d] q[i,d] / 64      (j on PSUM partitions)
  attn_T = exp(scores_T)  (scores are tiny: |s|<1, so no max-subtraction)
  out[i, 0:64] | l[i] = sum_j attn_T[j, i] * [v[j, :] | 1]
    computed per (j-chunk, i-chunk) 128x128 block with attn_T as the
    STATIONARY operand and [v|1] (65 cols) as the moving operand -> out in
    natural [i, d] layout, 65 moving cycles per block.
  out[i] /= l[i]
Causality: blocks with jc > ic are never computed (exp windows trimmed);
the 4 diagonal 128x128 blocks per i-tile are masked via affine_select.
"""

import numpy as np

import concourse.bacc as bacc
import concourse.bass as bass
import concourse.mybir as mybir
import concourse.tile as tile
from concourse.bass_utils import run_bass_kernel_spmd
from concourse.masks import make_identity

F32 = mybir.dt.float32
BF16 = mybir.dt.bfloat16

B, S, D = 16, 2048, 64
N_CORES = 8
BPC = B // N_CORES  # batches per core
P = 128
ITILE = 512               # i-tile width (free dim of scores_T)
N_IT = S // ITILE         # 4 i-tiles
N_JC = S // P             # 16 j-chunks
SCALE = 1.0 / D


import os as _os

QUAD_BUFS = int(_os.environ.get("K_QUAD_BUFS", "2"))
ACC_BUFS = int(_os.environ.get("K_ACC_BUFS", "2"))
TRP_BUFS = int(_os.environ.get("K_TRP_BUFS", "2"))
LAG_N = int(_os.environ.get("K_LAG", "4"))
INJ_N = int(_os.environ.get("K_INJ", "10"))
ATTN_BUFS = int(_os.environ.get("K_ATTN_BUFS", "4"))
NOPACK = int(_os.environ.get("K_NOPACK", "0"))  # timing-only A/B probe
MERGE = int(_os.environ.get("K_MERGE", "1"))  # one pipeline across batches


def build_kernel(loop: int = 0, level: int = 4):
    # level: probe ladder for benchmarking — 1: input DMA only, 2: + stage A,
    # 3: + mm1/exp/mask, 4: full kernel (default; the only correct one),
    # 5: stage A + all matmuls, no ACT/mask, 6: stage A + exp stream only
    nc = bacc.Bacc("TRN2", target_bir_lowering=False, debug=False)
    q_h = nc.dram_tensor("q", [BPC, S, D], F32, kind="ExternalInput").ap()
    k_h = nc.dram_tensor("k", [BPC, S, D], F32, kind="ExternalInput").ap()
    v_h = nc.dram_tensor("v", [BPC, S, D], F32, kind="ExternalInput").ap()
    o_h = nc.dram_tensor("o", [BPC, S, D], F32, kind="ExternalOutput").ap()

    with tile.TileContext(nc) as tc:
        with (
            tc.tile_pool(name="const", bufs=1) as const,
            tc.tile_pool(name="stage", bufs=2) as stage,
            tc.tile_pool(name="qkt", bufs=2) as qkt,
            tc.tile_pool(name="attn", bufs=ATTN_BUFS) as attnp,
            tc.tile_pool(name="outs", bufs=2) as outs,
            tc.tile_pool(name="quad", bufs=QUAD_BUFS, space="PSUM") as quadp,
            tc.tile_pool(name="acc", bufs=ACC_BUFS, space="PSUM") as accp,
            tc.tile_pool(name="trp", bufs=TRP_BUFS, space="PSUM") as trp,
        ):
            ident_f = const.tile([P, P], F32)
            make_identity(nc, ident_f)
            ident_b = const.tile([P, P], BF16)
            nc.vector.tensor_copy(ident_b, ident_f)
            # warm the ACT exp table while the input DMAs run
            warm = const.tile([P, 1], F32)
            nc.scalar.activation(
                warm, ident_f[:, 0:1], mybir.ActivationFunctionType.Exp
            )
            # causal masking constants: accumulating negdiag^T @ tri into a
            # diagonal score block adds -1e20 wherever i < j (strictly above
            # the diagonal), so exp() zeroes those weights with no cross-
            # engine mask op. negdiag = diag(-1e20), tri[j, i] = 1 iff i < j.
            NEG = -1.0e20
            negdiag = const.tile([P, P], BF16)
            nc.vector.scalar_tensor_tensor(
                negdiag, ident_f, NEG, ident_f,
                op0=mybir.AluOpType.mult, op1=mybir.AluOpType.mult,
            )
            tri = const.tile([P, P], BF16)
            nc.gpsimd.memset(tri, 1.0)
            nc.gpsimd.affine_select(
                out=tri, in_=tri,
                compare_op=mybir.AluOpType.is_ge,
                fill=0.0, base=-1, pattern=[[-1, P]], channel_multiplier=1,
            )

            def stage_a_loads(b):
                # ---- stage inputs, natural layout [128, 16, 64]
                # halves so DMA/transpose pipeline at half granularity
                H = N_JC // 2
                qn = stage.tile([P, N_JC, D], F32, tag="qn", name=f"qn{b}")
                kn = stage.tile([P, N_JC, D], F32, tag="kn", name=f"kn{b}")
                vn = stage.tile([P, N_JC, D], F32, tag="vn", name=f"vn{b}")
                vp = stage.tile([P, N_JC, D + 1], BF16, tag="vp", name=f"vp{b}")
                kr = k_h[b].rearrange("(n p) d -> p n d", p=P)
                qr = q_h[b].rearrange("(n p) d -> p n d", p=P)
                vr = v_h[b].rearrange("(n p) d -> p n d", p=P)
                Q = N_JC // 4
                # quarters of k/q interleaved (transposes consume in this
                # order), v halves placed just before their first use
                sched = [
                    (kr, kn, 0), (qr, qn, 0), (kr, kn, 1), (qr, qn, 1),
                    (vr, vn, None), (kr, kn, 2), (qr, qn, 2),
                    (kr, kn, 3), (qr, qn, 3), (vr, vn, None),
                ]
                vh = 0
                for src, dst, qi in sched:
                    if qi is None:
                        sl = slice(H * vh, H * (vh + 1))
                        nc.sync.dma_start(dst[:, sl, :], src[:, sl, :])
                        if level >= 2:
                            nc.gpsimd.tensor_copy(vp[:, sl, 0:D], vn[:, sl, :])
                        vh += 1
                    elif b == 0 and qi == 0:
                        # split the first k/q quarters into eighths so the
                        # first transposes (and the whole pipeline) start
                        # ~1us earlier
                        E = Q // 2
                        for e in range(2):
                            sl = slice(E * e, E * (e + 1))
                            nc.sync.dma_start(dst[:, sl, :], src[:, sl, :])
                    else:
                        sl = slice(Q * qi, Q * (qi + 1))
                        nc.sync.dma_start(dst[:, sl, :], src[:, sl, :])
                if level >= 2:
                    nc.gpsimd.memset(vp[:, :, D : D + 1], 1.0)
                return qn, kn, vn, vp

            def stage_a_pe(b, qn, kn):
                """Return (qt, kt2, thunks): each thunk emits one PE
                transpose group; caller decides where to interleave them."""
                # K^T interleaved-pairs layout [128, S/2] bf16:
                #  kt2[0:64,  128e+s] = K^T of chunk 2e
                #  kt2[64:128,128e+s] = K^T of chunk 2e+1
                # (transposing a [128, 128] block of TWO adjacent chunks puts
                # chunk 2e on partitions 0:64 and chunk 2e+1 on 64:128).
                # Transposes read the fp32 tiles; bf16 cast is folded into
                # the PSUM->SBUF copy.
                kt2 = qkt.tile([P, S // 2], BF16, tag="kt", name=f"kt{b}")
                qt = qkt.tile([P, S], BF16, tag="qt", name=f"qt{b}")
                thunks = []

                def k_group(g):
                    tr = trp.tile([P, 2 * P], F32, tag="trp", name=f"trk{b}_{g}")
                    for u in range(2):
                        e = 2 * g + u
                        nc.tensor.transpose(
                            tr[:, P * u : P * (u + 1)],
                            kn[:, 2 * e : 2 * e + 2, :],
                            ident_f,
                        )
                    nc.vector.tensor_copy(
                        kt2[:, 2 * P * g : 2 * P * (g + 1)], tr
                    )

                def q_group(g):
                    # Q^T duplicated into both partition halves (two copies
                    # from the same PSUM tile; partition-shifted second copy)
                    tr = trp.tile([P, 4 * P], F32, tag="trp", name=f"trq{b}_{g}")
                    for u in range(4):
                        nc.tensor.transpose(
                            tr[0:D, P * u : P * (u + 1)],
                            qn[:, 4 * g + u, :],
                            ident_f,
                        )
                    sl = slice(4 * P * g, 4 * P * (g + 1))
                    nc.vector.tensor_copy(qt[0:D, sl], tr[0:D])
                    nc.vector.tensor_copy(qt[D : 2 * D, sl], tr[0:D])

                # interleave k/q groups so the data the first matmuls need
                # (low j-chunks, low i-columns) is ready earliest
                for g in range(4):
                    thunks.append(lambda g=g: k_group(g))
                    thunks.append(lambda g=g: q_group(g))
                return qt, kt2, thunks

            def one_pass():
                loaded = [stage_a_loads(b) for b in range(BPC)]
                if level < 2:
                    nc.sync.dma_start(o_h[0, 0:P, :], ident_f[:, 0:D])
                    return
                pe_stage = [
                    stage_a_pe(b, loaded[b][0], loaded[b][1])
                    for b in range(BPC)
                ]
                staged = []
                for b in range(BPC):
                    qt, kt2, thunks = pe_stage[b]
                    staged.append((qt, kt2, loaded[b][3], thunks))
                if level < 3:
                    for _, _, _, thunks in staged:
                        for t in thunks:
                            t()
                    nc.sync.dma_start(o_h[0, 0:P, :], ident_f[:, 0:D])
                    return
                LAG = LAG_N  # mm2 trails mm1/exp by LAG pairs: PE (strict FIFO)
                #          must never queue an mm2 whose exp isn't done yet
                pairs = [
                    (it, pr) for it in range(N_IT) for pr in range(2 * (it + 1))
                ]
                INJ_START = INJ_N  # during batch b's main loop, emit batch b+1's
                #                 PE transpose groups starting at this pair

                def stage_norm(b, it, out_ps):
                    # out_ps [128, 4, 65]: i = 128*k + p; normalize and store
                    rec = outs.tile([P, N_IT], F32, tag="rec")
                    nc.vector.reciprocal(rec, out_ps[:, :, D])
                    fin = outs.tile([P, N_IT, D], F32, tag="fin")
                    nc.vector.tensor_tensor(
                        fin,
                        out_ps[:, :, 0:D],
                        rec[:, :, None].to_broadcast((P, N_IT, D)),
                        mybir.AluOpType.mult,
                    )
                    r0 = ITILE * it
                    nc.sync.dma_start(
                        o_h[b, r0 : r0 + ITILE, :].rearrange(
                            "(s p) d -> p s d", p=P
                        ),
                        fin,
                    )

                # one software pipeline across BOTH batches: no drain/refill
                # at the batch boundary. Batch 1 runs its i-tiles in
                # DESCENDING order so the pipeline drains on the smallest
                # tile (it=0: shortest mm2+norm tail after the last exp).
                tile_order = {0: list(range(N_IT)), 1: list(range(N_IT))[::-1]}
                gpairs = [
                    (b, it, pr)
                    for b in range(BPC)
                    for it in tile_order[b]
                    for pr in range(2 * (it + 1))
                ]
                npb = len(pairs)
                atc = None
                if level == 5:
                    atc = attnp.tile([P, 2, ITILE], BF16, tag="attn")
                    nc.gpsimd.memset(atc, 0.25)
                out_ps_by_key = {}
                at_by_idx = {}

                def do_mm2(idx):
                    # the acc bank holds all 4 i-chunk accumulators as ONE
                    # psum accumulation group: start only on the very first
                    # matmul of the i-tile (zeroes the whole 2KB bank), stop
                    # only on the very last.
                    b, it, pr = gpairs[idx]
                    vp = staged[b][2]
                    at = at_by_idx.pop(idx)
                    out_ps = out_ps_by_key[(b, it)]
                    for c in range(2):
                        jc = 2 * pr + c
                        kj = jc - 4 * it  # relative j-chunk in this i-tile
                        for kc in range(max(kj, 0), N_IT):
                            nc.tensor.matmul(
                                out_ps[:, kc, 0 : D + 1],
                                lhsT=at[:, c, P * kc : P * (kc + 1)],
                                rhs=vp[:, jc, :],
                                start=(pr == 0 and c == 0 and kc == 0),
                                stop=(c == 1 and kc == N_IT - 1
                                      and pr == 2 * it + 1),
                            )
                    if pr == 2 * it + 1:
                        stage_norm(b, it, out_ps_by_key.pop((b, it)))

                # prefetch batch 0's first transpose groups before the loop
                for _i in range(2):
                    staged[0][3].pop(0)()
                for idx, (b, it, pr) in enumerate(gpairs):
                    qt, kt2, vp, _ = staged[b]
                    if pr == 0:
                        # padded to a full 2KB psum bank so pool slots
                        # never share a zero region
                        out_ps_by_key[(b, it)] = accp.tile(
                            [P, N_IT, P], F32, tag="acc",
                            name=f"acc{b}_{it}",
                        )
                    st = quadp.tile([P, 2, ITILE], F32, tag="quad")
                    isl = slice(ITILE * it, ITILE * (it + 1))
                    diag = pr >= 2 * it  # diagonal pair: windows trimmed
                    at = None
                    if level != 5:
                        at = attnp.tile([P, 2, ITILE], BF16, tag="attn")
                    for c in range(2):
                        kj = 2 * pr + c - 4 * it
                        i0 = P * kj if diag else 0
                        # c=0 in PE row group 0, c=1 in row group 64:
                        # the two matmuls stream concurrently
                        h = slice(0, D) if NOPACK else slice(D * c, D * (c + 1))
                        mask_here = diag and level != 7
                        nc.tensor.matmul(
                            st[:, c, i0:ITILE],
                            lhsT=kt2[h, P * pr : P * (pr + 1)],
                            rhs=qt[h, ITILE * it + i0 : ITILE * (it + 1)],
                            start=True,
                            stop=not mask_here,
                        )
                        if mask_here:
                            # fold the causal mask into the scores on the PE:
                            # accumulate -1e20 above the diagonal of the
                            # diagonal 128x128 block; exp() then zeroes it
                            nc.tensor.matmul(
                                st[:, c, i0 : i0 + P],
                                lhsT=negdiag,
                                rhs=tri,
                                start=False,
                                stop=True,
                            )
                    if level != 5:
                        if diag:
                            for c in range(2):
                                # per-chunk trimmed exp window
                                i0 = P * (2 * pr + c - 4 * it)
                                nc.scalar.activation(
                                    at[:, c, i0:ITILE], st[:, c, i0:ITILE],
                                    mybir.ActivationFunctionType.Exp,
                                    scale=SCALE,
                                )
                        else:
                            nc.scalar.activation(
                                at, st, mybir.ActivationFunctionType.Exp,
                                scale=SCALE,
                            )
                    if b == 0 and pr == 2 * it + 1:
                        # prefetch the NEXT i-tile's k_group/q_group right
                        # after this tile's last mm1/exp is emitted
                        th = staged[0][3]
                        for _i in range(2):
                            if th:
                                th.pop(0)()
                    if level == 5:
                        at_by_idx[idx] = atc
                    else:
                        at_by_idx[idx] = at
                    if level == 3:
                        at_by_idx.pop(idx)
                        continue
                    if idx >= LAG and (MERGE or idx % npb >= LAG):
                        do_mm2(idx - LAG)
                    if not MERGE and idx % npb == npb - 1:
                        # flush this batch's lagged mm2s before the next batch
                        for j in range(idx - LAG + 1, idx + 1):
                            do_mm2(j)
                    # inject next batch's transpose groups into this batch's
                    # main loop (PE FIFO: their input DMAs are long done)
                    bi = idx // npb
                    if idx % npb >= INJ_START and bi + 1 < BPC:
                        th = staged[bi + 1][3]
                        if th:
                            th.pop(0)()
                if level != 3 and MERGE:
                    for idx in range(len(gpairs) - LAG, len(gpairs)):
                        do_mm2(idx)
                for b in range(BPC):
                    for t in staged[b][3]:
                        t()
                if level < 4:
                    nc.sync.dma_start(o_h[0, 0:P, :], ident_f[:, 0:D])

            if loop:
                hints = ()
                if _os.environ.get("K_LOOP_HINTS"):
                    hints = (
                        mybir.EngineType.PE,
                        mybir.EngineType.Activation,
                        mybir.EngineType.DVE,
                        mybir.EngineType.Pool,
                        mybir.EngineType.SP,
                    )
                with tc.For_i(0, loop, 1, hint_engines=hints):
                    one_pass()
            else:
                one_pass()

    nc.compile()
    return nc


_CACHE: dict = {}


def _get_nc(loop: int = 0):
    if loop not in _CACHE:
        _CACHE[loop] = build_kernel(loop)
    return _CACHE[loop]


def kernel(q: np.ndarray, k: np.ndarray, v: np.ndarray) -> np.ndarray:
    q = np.ascontiguousarray(q, dtype=np.float32)
    k = np.ascontiguousarray(k, dtype=np.float32)
    v = np.ascontiguousarray(v, dtype=np.float32)
    nc = _get_nc(0)
    in_maps = [
        {
            "q": q[BPC * i : BPC * (i + 1)],
            "k": k[BPC * i : BPC * (i + 1)],
            "v": v[BPC * i : BPC * (i + 1)],
        }
        for i in range(N_CORES)
    ]
    res = run_bass_kernel_spmd(nc, in_maps, list(range(N_CORES)))
    return np.concatenate([res.results[i]["o"] for i in range(N_CORES)], axis=0)
